# revision 18
# baseline (speedup 1.0000x reference)
"""CRF-as-RNN mean-field kernel for Trainium2 (Bass/Tile), 8-core SPMD.

Strategy:
- Shard 2 images x 4 row-strips across 8 cores. Each core gets 84 rows
  (64 owned + halo); 5 mean-field iterations shrink the valid region by
  2 rows/iter, so no inter-core communication is needed at all.
- On-chip layout: partitions = 6 row-groups x 21 channels = 126; free dim
  = 14 rows x 256 cols (+2-row/-col halos for in-tile shifted reads:
  q has 18 row-slots x 260 col-slots; w maps 16 row-slots). Image-boundary
  zero padding is realized by statically-zero halo slots; intra-core group
  halos are refreshed once per iteration with two SBUF->SBUF DMAs.
- The 5x5 spatial gaussian (sigma=0.1) is a numerical delta in f32, so
  sp == q; it is folded into the compat matmul: z += (1+wc)*mneg @ q.
- Bilateral 24-tap MAC on DVE in fp16 2x mode; 12 unique weight maps
  serve opposite tap pairs by symmetry. (GPSIMD offload was tried and
  reverted: concurrent GPSIMD+DVE streams contend ~4x on SBUF.)
- Bilateral is computed in two row-blocks (rows 0-5, rows 6-13) and the
  softmax is chunked [0,1,2,6,3,4,5] so the next iteration's block A
  only waits on chunks {3,6} + halo DMAs; softmax chunks 4,5 and 0..2
  overlap the bilateral DVE burst.
- w-map precompute: img arrives f16; diff/square on DVE at 2x; a single
  [18->126] f16 mask matmul kron(I6, ones(3,21)) does the 3-channel
  reduction AND the 21-channel broadcast in one op; ACT exp applies
  scale=-50 and folds the spatial weight via bias=ln(s_k).
- Softmax chunked through PSUM, all-f16 matmuls; lnD recentered by
  ln(21) (scale=1/21) so f16 holds it accurately; -lnD broadcast back
  into PSUM by a mask matmul; final exp carries bias=-ln(21).
  Output written f16, host upcasts.
"""

import math
import sys
from contextlib import ExitStack

import numpy as np

sys.path.insert(0, "/opt/trn_rl_repo")

# ---------------- problem constants (hardcoded per contract) ----------------
B, C, H, W = 2, 21, 256, 256
G, RG = 6, 14                  # row groups per strip, rows per group
P = G * C                      # 126 partitions
F = RG * W                     # 3584 free elems (real pixels per partition)
NT, NV = 18, 260               # q tile row slots (-2..15), col slots (-2..257)
NTW = 16                       # w/diff/sq tile row slots (-2..13)
IU, IV = 22, 264               # img tile row slots (-4..17), col slots (-4..259)
STARTS = [0, 54, 118, 172]     # strip start rows
OWN = [(0, 64), (10, 74), (10, 74), (20, 84)]  # owned local-row range per strip
NUM_ITERS = 5
NCH, CH = 7, 512               # softmax chunks (512 px = 2 rows)
CHUNK_ORDER = [0, 1, 2, 6, 3, 4, 5]
FW = NTW * NV                  # 4160 w-map free elems
PRE_CHUNKS = [(i * 512, 512) for i in range(8)] + [(4096, 64)]
LN21 = math.log(21.0)
ROW_BLOCKS = [(0, 6), (6, 14)]  # bilateral row blocks
# tap-instances whose product skips the DVE add chain and is instead
# accumulated into PSUM by an extra mneg@prod matmul per softmax chunk
# (chosen among qdy>0 instances, which are dependency-blocked early anyway)
PEI = (4, 6, 8, 10, 12, 16, 20, 22)

# spatial gaussian (5x5, sigma=5), normalized
_ax = np.arange(5, dtype=np.float64) - 2
_xx, _yy = np.meshgrid(_ax, _ax, indexing="ij")
_g = np.exp(-(_xx**2 + _yy**2) / (2 * 5.0**2))
SW = (_g / _g.sum()).astype(np.float64)
WC = float(SW[2, 2])           # center weight (spatial only; color=1 at center)
# 12 unique taps (positive half-window); opposite taps share weight maps
TAPS = [(0, 1), (0, 2), (1, -2), (1, -1), (1, 0), (1, 1), (1, 2),
        (2, -2), (2, -1), (2, 0), (2, 1), (2, 2)]

_BASS_CACHE = {}


def _build_bass():
    import concourse.bass as bass
    import concourse.mybir as mybir
    from concourse import tile

    f32 = mybir.dt.float32
    f16 = mybir.dt.float16
    AF = mybir.ActivationFunctionType

    nc = bass.Bass("TRN2", target_bir_lowering=False, debug=False,
                   enable_asserts=False)

    lg_d = nc.dram_tensor("lg", [P, F], f16, kind="ExternalInput")
    img_d = nc.dram_tensor("img", [18, IU * IV], f16, kind="ExternalInput")
    mneg_d = nc.dram_tensor("mneg", [P, P], f16, kind="ExternalInput")
    mneg2_d = nc.dram_tensor("mneg2", [P, P], f16, kind="ExternalInput")
    iden_d = nc.dram_tensor("iden", [P, P], f16, kind="ExternalInput")
    onesd_d = nc.dram_tensor("onesd", [P, G], f16, kind="ExternalInput")
    bneg_d = nc.dram_tensor("bneg", [G, P], f16, kind="ExternalInput")
    m18_d = nc.dram_tensor("m18", [18, P], f16, kind="ExternalInput")
    lnsw_d = nc.dram_tensor("lnsw", [P, 13], f32, kind="ExternalInput")
    qout_d = nc.dram_tensor("qout", [P, F], f16, kind="ExternalOutput")

    with tile.TileContext(nc) as tc, ExitStack() as ctx:
        const_pool = ctx.enter_context(tc.tile_pool(name="const", bufs=1))
        main_pool = ctx.enter_context(tc.tile_pool(name="main", bufs=1))
        w_pool = ctx.enter_context(tc.tile_pool(name="wmaps", bufs=1))

        pre_pool = ctx.enter_context(tc.tile_pool(name="pre0", bufs=1))
        img_t = pre_pool.tile([18, IU * IV], f16, tag="img")
        nc.sync.dma_start(img_t[:], img_d.ap())
        lg_t = pre_pool.tile([P, F], f16, tag="lg")
        nc.sync.dma_start(lg_t[:], lg_d.ap())
        mneg_t = const_pool.tile([P, P], f16, tag="mneg")
        nc.sync.dma_start(mneg_t[:], mneg_d.ap())
        mneg2_t = const_pool.tile([P, P], f16, tag="mneg2")
        nc.sync.dma_start(mneg2_t[:], mneg2_d.ap())
        iden_t = const_pool.tile([P, P], f16, tag="iden")
        nc.sync.dma_start(iden_t[:], iden_d.ap())
        onesd_t = const_pool.tile([P, G], f16, tag="onesd")
        nc.sync.dma_start(onesd_t[:], onesd_d.ap())
        bneg_t = const_pool.tile([G, P], f16, tag="bneg")
        nc.sync.dma_start(bneg_t[:], bneg_d.ap())
        m18_t = const_pool.tile([18, P], f16, tag="m18")
        nc.sync.dma_start(m18_t[:], m18_d.ap())
        lnsw_t = const_pool.tile([P, 13], f32, tag="lnsw")
        nc.sync.dma_start(lnsw_t[:], lnsw_d.ap())

        # Absorber matmuls: each PE matmul can carry only ~1 sync wait
        # beyond its own-engine wait, so pre-observe every stationary's DMA
        # queue with a 2-column dummy matmul (self-referential rhs => the
        # dummy itself waits on exactly one DMA sem).
        with tc.tile_pool(name="scrp", bufs=1, space="PSUM") as scrp:
            scr = scrp.tile([G, 2], f32, tag="scr")
            nc.tensor.matmul(scr[:1, :], mneg_t[:, 0:1], mneg_t[:, 0:2],
                             start=True, stop=True)
            nc.tensor.matmul(scr[:1, :], mneg2_t[:, 0:1], mneg2_t[:, 0:2],
                             start=True, stop=True)
            nc.tensor.matmul(scr[:1, :], iden_t[:, 0:1], iden_t[:, 0:2],
                             start=True, stop=True)
            nc.tensor.matmul(scr[:, :], onesd_t[:], onesd_t[:, 0:2],
                             start=True, stop=True)
            nc.tensor.matmul(scr[:1, :], bneg_t[:, 0:1], bneg_t[:, 0:2],
                             start=True, stop=True)
            nc.tensor.matmul(scr[:1, :], m18_t[:, 0:1], m18_t[:, 0:2],
                             start=True, stop=True)

        q_t = main_pool.tile([P, NT * NV], f16, tag="q")
        q3 = q_t[:].rearrange("p (t v) -> p t v", v=NV)
        # only halo slots need zeroing; the interior is written by the
        # initial softmax before any bilateral read
        nc.vector.memset(q3[:, 0:2, 0:NV], 0.0)
        nc.vector.memset(q3[:, 16:18, 0:NV], 0.0)
        nc.vector.memset(q3[:, 2:16, 0:2], 0.0)
        nc.vector.memset(q3[:, 2:16, 258:260], 0.0)

        w_tiles = [w_pool.tile([P, FW], f16, tag=f"w{i}", name=f"w{i}")
                   for i in range(len(TAPS))]

        zps_pool = ctx.enter_context(tc.tile_pool(name="zps", bufs=4,
                                                  space="PSUM"))
        dps_pool = ctx.enter_context(tc.tile_pool(name="dps", bufs=2,
                                                  space="PSUM"))

        # ---------------- iteration tiles ----------------
        post_pool = ctx.enter_context(tc.tile_pool(name="post", bufs=1))
        acca_t = post_pool.tile([P, F], f16, tag="acca")
        acca3 = acca_t[:].rearrange("p (r x) -> p r x", x=W)
        qo3 = acca3  # final pass writes into acca (dead by then)
        prod_ts = []  # filled after the precompute pool closes (reuses SBUF)
        tmp_pool = ctx.enter_context(tc.tile_pool(name="tmp", bufs=3))
        e_pool = ctx.enter_context(tc.tile_pool(name="E", bufs=2))
        ln_pool = ctx.enter_context(tc.tile_pool(name="ln", bufs=2))

        def sm_chunk(c, with_s, last):
            use_prods = with_s
            sl = slice(c * CH, (c + 1) * CH)
            z_ps = zps_pool.tile([P, CH], f32, tag="z")
            if with_s:
                nc.tensor.matmul(z_ps[:], mneg_t[:], acca_t[:, sl],
                                 start=True, stop=False)
                if use_prods:
                    for pt in prod_ts:
                        nc.tensor.matmul(z_ps[:], mneg_t[:], pt[:, sl],
                                         start=False, stop=False,
                                         skip_group_check=True)
                nc.tensor.matmul(z_ps[:], mneg2_t[:],
                                 q3[:, 2 + 2 * c:4 + 2 * c, 2:2 + W],
                                 start=False, stop=False,
                                 skip_group_check=True)
                nc.tensor.matmul(z_ps[:], iden_t[:], lg_t[:, sl],
                                 start=False, stop=False,
                                 skip_group_check=True)
            else:
                nc.tensor.matmul(z_ps[:], iden_t[:], lg_t[:, sl],
                                 start=True, stop=False,
                                 skip_group_check=True)
            e_t = e_pool.tile([P, CH], f16, tag="E")
            nc.scalar.activation(e_t[:], z_ps[:], AF.Exp)
            d_ps = dps_pool.tile([G, CH], f32, tag="D")
            nc.tensor.matmul(d_ps[:], onesd_t[:], e_t[:],
                             start=True, stop=True)
            ln_t = ln_pool.tile([G, CH], f16, tag="ln")
            # ln(D/21) stays ~O(1) => accurate in f16
            nc.scalar.activation(ln_t[:], d_ps[:], AF.Ln,
                                 scale=float(1.0 / 21.0))
            nc.tensor.matmul(z_ps[:], bneg_t[:], ln_t[:],
                             start=False, stop=True,
                             skip_group_check=True)
            z3 = z_ps[:].rearrange("p (r x) -> p r x", x=W)
            if last:
                nc.scalar.activation(qo3[:, 2 * c:2 * c + 2, 0:W],
                                     z3, AF.Exp, bias=lnsw_t[:, 12:13])
                nc.sync.dma_start(qout_d.ap()[:, sl], acca_t[:, sl])
            else:
                nc.scalar.activation(
                    q3[:, 2 + 2 * c:4 + 2 * c, 2:2 + W], z3, AF.Exp,
                    bias=lnsw_t[:, 12:13])

        def softmax_pass(with_s, last):
            for c in CHUNK_ORDER:
                sm_chunk(c, with_s, last)

        def bilateral_block(r0, r1):
            rw = (r1 - r0) * W
            fsl = slice(r0 * W, r1 * W)
            first_d = True
            idx = 0
            for ki, (dy, dx) in enumerate(TAPS):
                w3 = w_tiles[ki][:].rearrange("p (t v) -> p t v", v=NV)
                for (qdy, qdx, wdy, wdx) in ((dy, dx, 0, 0),
                                             (-dy, -dx, -dy, -dx)):
                    q_ap = q3[:, 2 + qdy + r0:2 + qdy + r1,
                              2 + qdx:2 + qdx + W]
                    w_ap = w3[:, 2 + wdy + r0:2 + wdy + r1,
                              2 + wdx:2 + wdx + W]
                    if idx in PEI:
                        pt = prod_ts[PEI.index(idx)]
                        p3 = pt[:].rearrange("p (r x) -> p r x", x=W)
                        nc.vector.tensor_mul(p3[:, r0:r1, 0:W], q_ap, w_ap)
                    elif first_d:
                        nc.vector.tensor_mul(acca3[:, r0:r1, 0:W],
                                             q_ap, w_ap)
                        first_d = False
                    else:
                        t = tmp_pool.tile([P, 8 * W], f16, tag="t")
                        t3 = t[:, 0:rw].rearrange("p (r x) -> p r x", x=W)
                        nc.vector.tensor_mul(t3, q_ap, w_ap)
                        nc.vector.tensor_add(acca_t[:, fsl],
                                             acca_t[:, fsl], t[:, 0:rw])
                    idx += 1

        # ---------------- w-map precompute (init softmax issued after tap 1
        # so tap-0/1 PE matmuls recycle the diff buffers quickly; the PE/ACT
        # precompute stream hides under DVE sub/square work and the first
        # iteration's bilateral; tap ki only needs w_ki) ----------
        with tc.tile_pool(name="pre", bufs=2) as prep, \
             tc.tile_pool(name="psp", bufs=2, space="PSUM") as psp:
            img3 = img_t[:].rearrange("p (u v) -> p u v", v=IV)

            for ki, (dy, dx) in enumerate(TAPS):
                if ki == 2:
                    softmax_pass(with_s=False, last=False)  # q0
                diff_t = prep.tile([18, FW], f16, tag="diff")
                diff3 = diff_t[:].rearrange("p (t v) -> p t v", v=NV)
                nc.vector.tensor_sub(
                    diff3[:, 0:NTW, 0:NV],
                    img3[:, 2 + dy:2 + dy + NTW, 2 + dx:2 + dx + NV],
                    img3[:, 2:2 + NTW, 2:2 + NV],
                )
                if ki % 2 == 0:
                    nc.vector.tensor_mul(diff_t[:], diff_t[:], diff_t[:])
                else:
                    nc.scalar.square(diff_t[:], diff_t[:])
                for c0, cw in PRE_CHUNKS:
                    sl = slice(c0, c0 + cw)
                    d2_ps = psp.tile([P, 512], f32, tag="d2")
                    nc.tensor.matmul(d2_ps[:, 0:cw], m18_t[:], diff_t[:, sl],
                                     start=True, stop=True)
                    nc.scalar.activation(w_tiles[ki][:, sl], d2_ps[:, 0:cw],
                                         AF.Exp, scale=-50.0,
                                         bias=lnsw_t[:, ki:ki + 1])

        post2_pool = ctx.enter_context(tc.tile_pool(name="post2", bufs=1))
        prod_ts.extend(
            post2_pool.tile([P, F], f16, tag=f"prod{j}", name=f"prod{j}")
            for j in range(len(PEI)))


        for it in range(NUM_ITERS):
            last = it == NUM_ITERS - 1
            # refresh intra-core group halos (2 SBUF->SBUF DMAs)
            nc.sync.dma_start(q3[21:126, 0:2, 0:NV], q3[0:105, 14:16, 0:NV])
            nc.sync.dma_start(q3[0:105, 16:18, 0:NV], q3[21:126, 2:4, 0:NV])

            for r0, r1 in ROW_BLOCKS:
                bilateral_block(r0, r1)

            softmax_pass(with_s=True, last=last)

    _legalize_matmul_waits(nc, mybir)
    return nc


def _legalize_matmul_waits(nc, mybir, max_waits=2):
    """TRN2 ISA sync-wait structs hold few waits per instruction (2 for PE
    matmult/NoOp, 1 for DVE TensorTensor, ...); codegen aborts on more.
    Move excess waits onto InstNoOps (1 wait each) inserted right before
    on the same engine (adjacent => identical blocking semantics)."""
    cap = {}
    for f in nc.m.functions:
        for blk in f.blocks:
            insts = blk.instructions
            out = []
            changed = False
            for i in insts:
                si = getattr(i, "sync_info", None)
                eng = getattr(i, "engine", None)
                max_waits = cap.get(type(i).__name__, 1)
                if (si is not None and eng is not None
                        and len(si.on_wait) > max_waits):
                    waits = list(si.on_wait)
                    keep, move = [], []
                    for w in waits:
                        if "PE" in w.ant_name and len(keep) < max_waits:
                            keep.append(w)
                        else:
                            move.append(w)
                    while len(keep) < max_waits and move:
                        keep.append(move.pop())
                    nop_cap = cap.get("InstNoOp", 1)
                    while move:
                        grp, move = move[:nop_cap], move[nop_cap:]
                        nop = mybir.InstNoOp(
                            name=nc.get_next_instruction_name(),
                            engine=eng, ins=[], outs=[])
                        nop.sync_info = mybir.SyncInfo(on_wait=grp,
                                                       on_update=[])
                        out.append(nop)
                    i.sync_info = mybir.SyncInfo(
                        on_wait=keep, on_update=list(si.on_update))
                    changed = True
                out.append(i)
            if changed:
                blk.instructions = out


def _prep_shards(logits, img, compat):
    """Host-side shard prep -> list of 8 in_maps."""
    mneg = np.kron(np.eye(G), -compat.T.astype(np.float64)).astype(np.float16)
    mneg2 = ((1.0 + WC) *
             np.kron(np.eye(G), -compat.T.astype(np.float64))
             ).astype(np.float16)
    iden = np.eye(P, dtype=np.float16)
    onesd = np.kron(np.eye(G), np.ones((C, 1))).astype(np.float16)
    bneg = np.kron(np.eye(G), -np.ones((1, C))).astype(np.float16)
    m18 = np.kron(np.eye(G), np.ones((3, C))).astype(np.float16)
    lnsw = np.zeros((P, 13), np.float32)
    for ki, (dy, dx) in enumerate(TAPS):
        lnsw[:, ki] = math.log(SW[2 + dy, 2 + dx])
    lnsw[:, 12] = -LN21

    in_maps = []
    for core in range(8):
        b, j = divmod(core, 4)
        s = STARTS[j]
        lg = logits[b, :, s:s + 84, :].reshape(C, G, RG, W)
        lg = np.ascontiguousarray(
            lg.transpose(1, 0, 2, 3).reshape(P, F)).astype(np.float16)
        im = np.zeros((G, 3, IU, IV), np.float16)
        for g in range(G):
            base = s + g * RG - 4
            u0, u1 = max(0, -base), min(IU, H - base)
            im[g, :, u0:u1, 4:4 + W] = img[b, :, base + u0:base + u1, :]
        im = im.reshape(18, IU * IV)
        in_maps.append({
            "lg": lg, "img": np.ascontiguousarray(im),
            "mneg": mneg, "mneg2": mneg2, "iden": iden, "onesd": onesd,
            "bneg": bneg, "m18": m18, "lnsw": lnsw,
        })
    return in_maps


def kernel(**inputs):
    logits = np.asarray(inputs["logits"], dtype=np.float32)
    img = np.asarray(inputs["img"], dtype=np.float32)
    compat = np.asarray(inputs["compat_mat"], dtype=np.float32)

    from concourse.bass_utils import run_bass_kernel_spmd

    if "nc" not in _BASS_CACHE:
        _BASS_CACHE["nc"] = _build_bass()
    nc = _BASS_CACHE["nc"]

    in_maps = _prep_shards(logits, img, compat)
    res = run_bass_kernel_spmd(nc, in_maps, core_ids=list(range(8)))
    _BASS_CACHE["last_result"] = res

    out = np.zeros((B, C, H, W), np.float32)
    for core in range(8):
        b, j = divmod(core, 4)
        s = STARTS[j]
        lo, hi = OWN[j]
        qc = res.results[core]["qout"].astype(np.float32).reshape(G, C, RG, W)
        qc = qc.transpose(1, 0, 2, 3).reshape(C, 84, W)
        out[b, :, s + lo:s + hi, :] = qc[:, lo:hi, :]
    return out


# revision 19
# speedup vs baseline: 1.0171x; 1.0171x over previous
"""CRF-as-RNN mean-field kernel for Trainium2 (Bass/Tile), 8-core SPMD.

Strategy:
- Shard 2 images x 4 row-strips across 8 cores. Each core gets 84 rows
  (64 owned + halo); 5 mean-field iterations shrink the valid region by
  2 rows/iter, so no inter-core communication is needed at all.
- On-chip layout: partitions = 6 row-groups x 21 channels = 126; free dim
  = 14 rows x 256 cols (+2-row/-col halos for in-tile shifted reads:
  q has 18 row-slots x 260 col-slots; w maps 16 row-slots). Image-boundary
  zero padding is realized by statically-zero halo slots; intra-core group
  halos are refreshed once per iteration with two SBUF->SBUF DMAs.
- The 5x5 spatial gaussian (sigma=0.1) is a numerical delta in f32, so
  sp == q; it is folded into the compat matmul: z += (1+wc)*mneg @ q.
- Bilateral 24-tap MAC on DVE in fp16 2x mode; 12 unique weight maps
  serve opposite tap pairs by symmetry. (GPSIMD offload was tried and
  reverted: concurrent GPSIMD+DVE streams contend ~4x on SBUF.)
- Bilateral is computed in two row-blocks (rows 0-5, rows 6-13) and the
  softmax is chunked [0,1,2,6,3,4,5] so the next iteration's block A
  only waits on chunks {3,6} + halo DMAs; softmax chunks 4,5 and 0..2
  overlap the bilateral DVE burst.
- w-map precompute: img arrives f16; diff/square on DVE at 2x; a single
  [18->126] f16 mask matmul kron(I6, ones(3,21)) does the 3-channel
  reduction AND the 21-channel broadcast in one op; ACT exp applies
  scale=-50 and folds the spatial weight via bias=ln(s_k).
- Softmax chunked through PSUM, all-f16 matmuls; lnD recentered by
  ln(21) (scale=1/21) so f16 holds it accurately; -lnD broadcast back
  into PSUM by a mask matmul; final exp carries bias=-ln(21).
  Output written f16, host upcasts.
"""

import math
import sys
from contextlib import ExitStack

import numpy as np

sys.path.insert(0, "/opt/trn_rl_repo")

# ---------------- problem constants (hardcoded per contract) ----------------
B, C, H, W = 2, 21, 256, 256
G, RG = 6, 14                  # row groups per strip, rows per group
P = G * C                      # 126 partitions
F = RG * W                     # 3584 free elems (real pixels per partition)
NT, NV = 18, 260               # q tile row slots (-2..15), col slots (-2..257)
NTW = 16                       # w/diff/sq tile row slots (-2..13)
IU, IV = 22, 264               # img tile row slots (-4..17), col slots (-4..259)
STARTS = [0, 54, 118, 172]     # strip start rows
OWN = [(0, 64), (10, 74), (10, 74), (20, 84)]  # owned local-row range per strip
NUM_ITERS = 5
NCH, CH = 7, 512               # softmax chunks (512 px = 2 rows)
CHUNK_ORDER = [0, 1, 2, 6, 3, 4, 5]
FW = NTW * NV                  # 4160 w-map free elems
PRE_CHUNKS = [(i * 512, 512) for i in range(8)] + [(4096, 64)]
LN21 = math.log(21.0)
ROW_BLOCKS = [(0, 6), (6, 14)]  # bilateral row blocks
# tap-instances whose product skips the DVE add chain and is instead
# accumulated into PSUM by an extra mneg@prod matmul per softmax chunk
# (chosen among qdy>0 instances, which are dependency-blocked early anyway)
PEI = (4, 6, 8, 10, 12, 16, 20, 22)

# spatial gaussian (5x5, sigma=5), normalized
_ax = np.arange(5, dtype=np.float64) - 2
_xx, _yy = np.meshgrid(_ax, _ax, indexing="ij")
_g = np.exp(-(_xx**2 + _yy**2) / (2 * 5.0**2))
SW = (_g / _g.sum()).astype(np.float64)
WC = float(SW[2, 2])           # center weight (spatial only; color=1 at center)
# 12 unique taps (positive half-window); opposite taps share weight maps
TAPS = [(0, 1), (0, 2), (1, -2), (1, -1), (1, 0), (1, 1), (1, 2),
        (2, -2), (2, -1), (2, 0), (2, 1), (2, 2)]

_BASS_CACHE = {}


def _build_bass():
    import concourse.bass as bass
    import concourse.mybir as mybir
    from concourse import tile

    f32 = mybir.dt.float32
    f16 = mybir.dt.float16
    AF = mybir.ActivationFunctionType

    nc = bass.Bass("TRN2", target_bir_lowering=False, debug=False,
                   enable_asserts=False)

    lg_d = nc.dram_tensor("lg", [P, F], f16, kind="ExternalInput")
    img_d = nc.dram_tensor("img", [18, IU * IV], f16, kind="ExternalInput")
    mneg_d = nc.dram_tensor("mneg", [P, P], f16, kind="ExternalInput")
    mneg2_d = nc.dram_tensor("mneg2", [P, P], f16, kind="ExternalInput")
    iden_d = nc.dram_tensor("iden", [P, P], f16, kind="ExternalInput")
    onesd_d = nc.dram_tensor("onesd", [P, G], f16, kind="ExternalInput")
    bneg_d = nc.dram_tensor("bneg", [G, P], f16, kind="ExternalInput")
    m18_d = nc.dram_tensor("m18", [18, P], f16, kind="ExternalInput")
    lnsw_d = nc.dram_tensor("lnsw", [P, 13], f32, kind="ExternalInput")
    qout_d = nc.dram_tensor("qout", [P, F], f16, kind="ExternalOutput")

    with tile.TileContext(nc) as tc, ExitStack() as ctx:
        const_pool = ctx.enter_context(tc.tile_pool(name="const", bufs=1))
        main_pool = ctx.enter_context(tc.tile_pool(name="main", bufs=1))
        w_pool = ctx.enter_context(tc.tile_pool(name="wmaps", bufs=1))

        pre_pool = ctx.enter_context(tc.tile_pool(name="pre0", bufs=1))
        img_t = pre_pool.tile([18, IU * IV], f16, tag="img")
        nc.sync.dma_start(img_t[:], img_d.ap())
        lg_t = pre_pool.tile([P, F], f16, tag="lg")
        nc.sync.dma_start(lg_t[:], lg_d.ap())
        mneg_t = const_pool.tile([P, P], f16, tag="mneg")
        nc.sync.dma_start(mneg_t[:], mneg_d.ap())
        mneg2_t = const_pool.tile([P, P], f16, tag="mneg2")
        nc.sync.dma_start(mneg2_t[:], mneg2_d.ap())
        iden_t = const_pool.tile([P, P], f16, tag="iden")
        nc.sync.dma_start(iden_t[:], iden_d.ap())
        onesd_t = const_pool.tile([P, G], f16, tag="onesd")
        nc.sync.dma_start(onesd_t[:], onesd_d.ap())
        bneg_t = const_pool.tile([G, P], f16, tag="bneg")
        nc.sync.dma_start(bneg_t[:], bneg_d.ap())
        m18_t = const_pool.tile([18, P], f16, tag="m18")
        nc.sync.dma_start(m18_t[:], m18_d.ap())
        lnsw_t = const_pool.tile([P, 13], f32, tag="lnsw")
        nc.sync.dma_start(lnsw_t[:], lnsw_d.ap())

        # Absorber matmuls: each PE matmul can carry only ~1 sync wait
        # beyond its own-engine wait, so pre-observe every stationary's DMA
        # queue with a 2-column dummy matmul (self-referential rhs => the
        # dummy itself waits on exactly one DMA sem).
        with tc.tile_pool(name="scrp", bufs=1, space="PSUM") as scrp:
            scr = scrp.tile([G, 2], f32, tag="scr")
            nc.tensor.matmul(scr[:1, :], mneg_t[:, 0:1], mneg_t[:, 0:2],
                             start=True, stop=True)
            nc.tensor.matmul(scr[:1, :], mneg2_t[:, 0:1], mneg2_t[:, 0:2],
                             start=True, stop=True)
            nc.tensor.matmul(scr[:1, :], iden_t[:, 0:1], iden_t[:, 0:2],
                             start=True, stop=True)
            nc.tensor.matmul(scr[:, :], onesd_t[:], onesd_t[:, 0:2],
                             start=True, stop=True)
            nc.tensor.matmul(scr[:1, :], bneg_t[:, 0:1], bneg_t[:, 0:2],
                             start=True, stop=True)
            nc.tensor.matmul(scr[:1, :], m18_t[:, 0:1], m18_t[:, 0:2],
                             start=True, stop=True)

        q_t = main_pool.tile([P, NT * NV], f16, tag="q")
        q3 = q_t[:].rearrange("p (t v) -> p t v", v=NV)
        # only halo slots need zeroing; the interior is written by the
        # initial softmax before any bilateral read
        nc.vector.memset(q3[:, 0:2, 0:NV], 0.0)
        nc.vector.memset(q3[:, 16:18, 0:NV], 0.0)
        nc.vector.memset(q3[:, 2:16, 0:2], 0.0)
        nc.vector.memset(q3[:, 2:16, 258:260], 0.0)

        w_tiles = [w_pool.tile([P, FW], f16, tag=f"w{i}", name=f"w{i}")
                   for i in range(len(TAPS))]

        zps_pool = ctx.enter_context(tc.tile_pool(name="zps", bufs=3,
                                                  space="PSUM"))
        dps_pool = ctx.enter_context(tc.tile_pool(name="dps", bufs=2,
                                                  space="PSUM"))

        # ---------------- iteration tiles ----------------
        post_pool = ctx.enter_context(tc.tile_pool(name="post", bufs=1))
        acca_t = post_pool.tile([P, F], f16, tag="acca")
        acca3 = acca_t[:].rearrange("p (r x) -> p r x", x=W)
        qo3 = acca3  # final pass writes into acca (dead by then)
        prod_ts = []  # filled after the precompute pool closes (reuses SBUF)
        tmp_pool = ctx.enter_context(tc.tile_pool(name="tmp", bufs=3))
        e_pool = ctx.enter_context(tc.tile_pool(name="E", bufs=2))
        ln_pool = ctx.enter_context(tc.tile_pool(name="ln", bufs=2))

        def sm_chunk(c, with_s, last):
            use_prods = with_s
            sl = slice(c * CH, (c + 1) * CH)
            z_ps = zps_pool.tile([P, CH], f32, tag="z")
            if with_s:
                nc.tensor.matmul(z_ps[:], mneg_t[:], acca_t[:, sl],
                                 start=True, stop=False)
                if use_prods:
                    for pt in prod_ts:
                        nc.tensor.matmul(z_ps[:], mneg_t[:], pt[:, sl],
                                         start=False, stop=False,
                                         skip_group_check=True)
                nc.tensor.matmul(z_ps[:], mneg2_t[:],
                                 q3[:, 2 + 2 * c:4 + 2 * c, 2:2 + W],
                                 start=False, stop=False,
                                 skip_group_check=True)
                nc.tensor.matmul(z_ps[:], iden_t[:], lg_t[:, sl],
                                 start=False, stop=False,
                                 skip_group_check=True)
            else:
                nc.tensor.matmul(z_ps[:], iden_t[:], lg_t[:, sl],
                                 start=True, stop=False,
                                 skip_group_check=True)
            e_t = e_pool.tile([P, CH], f16, tag="E")
            nc.scalar.activation(e_t[:], z_ps[:], AF.Exp)
            d_ps = dps_pool.tile([G, CH], f32, tag="D")
            nc.tensor.matmul(d_ps[:], onesd_t[:], e_t[:],
                             start=True, stop=True)
            ln_t = ln_pool.tile([G, CH], f16, tag="ln")
            # ln(D/21) stays ~O(1) => accurate in f16
            nc.scalar.activation(ln_t[:], d_ps[:], AF.Ln,
                                 scale=float(1.0 / 21.0))
            nc.tensor.matmul(z_ps[:], bneg_t[:], ln_t[:],
                             start=False, stop=True,
                             skip_group_check=True)
            z3 = z_ps[:].rearrange("p (r x) -> p r x", x=W)
            if last:
                nc.scalar.activation(qo3[:, 2 * c:2 * c + 2, 0:W],
                                     z3, AF.Exp, bias=lnsw_t[:, 12:13])
                nc.sync.dma_start(qout_d.ap()[:, sl], acca_t[:, sl])
            else:
                nc.scalar.activation(
                    q3[:, 2 + 2 * c:4 + 2 * c, 2:2 + W], z3, AF.Exp,
                    bias=lnsw_t[:, 12:13])

        def softmax_pass(with_s, last):
            for c in CHUNK_ORDER:
                sm_chunk(c, with_s, last)

        def bilateral_block(r0, r1):
            rw = (r1 - r0) * W
            fsl = slice(r0 * W, r1 * W)
            first_d = True
            idx = 0
            for ki, (dy, dx) in enumerate(TAPS):
                w3 = w_tiles[ki][:].rearrange("p (t v) -> p t v", v=NV)
                for (qdy, qdx, wdy, wdx) in ((dy, dx, 0, 0),
                                             (-dy, -dx, -dy, -dx)):
                    q_ap = q3[:, 2 + qdy + r0:2 + qdy + r1,
                              2 + qdx:2 + qdx + W]
                    w_ap = w3[:, 2 + wdy + r0:2 + wdy + r1,
                              2 + wdx:2 + wdx + W]
                    if idx in PEI:
                        pt = prod_ts[PEI.index(idx)]
                        p3 = pt[:].rearrange("p (r x) -> p r x", x=W)
                        nc.vector.tensor_mul(p3[:, r0:r1, 0:W], q_ap, w_ap)
                    elif first_d:
                        nc.vector.tensor_mul(acca3[:, r0:r1, 0:W],
                                             q_ap, w_ap)
                        first_d = False
                    else:
                        t = tmp_pool.tile([P, 8 * W], f16, tag="t")
                        t3 = t[:, 0:rw].rearrange("p (r x) -> p r x", x=W)
                        nc.vector.tensor_mul(t3, q_ap, w_ap)
                        nc.vector.tensor_add(acca_t[:, fsl],
                                             acca_t[:, fsl], t[:, 0:rw])
                    idx += 1

        # ---------------- w-map precompute (init softmax issued after tap 1
        # so tap-0/1 PE matmuls recycle the diff buffers quickly; the PE/ACT
        # precompute stream hides under DVE sub/square work and the first
        # iteration's bilateral; tap ki only needs w_ki) ----------
        with tc.tile_pool(name="pre", bufs=2) as prep, \
             tc.tile_pool(name="psp", bufs=3, space="PSUM") as psp:
            img3 = img_t[:].rearrange("p (u v) -> p u v", v=IV)

            for ki, (dy, dx) in enumerate(TAPS):
                if ki == 2:
                    softmax_pass(with_s=False, last=False)  # q0
                diff_t = prep.tile([18, FW], f16, tag="diff")
                diff3 = diff_t[:].rearrange("p (t v) -> p t v", v=NV)
                nc.vector.tensor_sub(
                    diff3[:, 0:NTW, 0:NV],
                    img3[:, 2 + dy:2 + dy + NTW, 2 + dx:2 + dx + NV],
                    img3[:, 2:2 + NTW, 2:2 + NV],
                )
                if ki % 2 == 0:
                    nc.vector.tensor_mul(diff_t[:], diff_t[:], diff_t[:])
                else:
                    nc.scalar.square(diff_t[:], diff_t[:])
                for c0, cw in PRE_CHUNKS:
                    sl = slice(c0, c0 + cw)
                    d2_ps = psp.tile([P, 512], f32, tag="d2")
                    nc.tensor.matmul(d2_ps[:, 0:cw], m18_t[:], diff_t[:, sl],
                                     start=True, stop=True)
                    nc.scalar.activation(w_tiles[ki][:, sl], d2_ps[:, 0:cw],
                                         AF.Exp, scale=-50.0,
                                         bias=lnsw_t[:, ki:ki + 1])

        post2_pool = ctx.enter_context(tc.tile_pool(name="post2", bufs=1))
        prod_ts.extend(
            post2_pool.tile([P, F], f16, tag=f"prod{j}", name=f"prod{j}")
            for j in range(len(PEI)))


        for it in range(NUM_ITERS):
            last = it == NUM_ITERS - 1
            # refresh intra-core group halos (2 SBUF->SBUF DMAs)
            nc.sync.dma_start(q3[21:126, 0:2, 0:NV], q3[0:105, 14:16, 0:NV])
            nc.sync.dma_start(q3[0:105, 16:18, 0:NV], q3[21:126, 2:4, 0:NV])

            for r0, r1 in ROW_BLOCKS:
                bilateral_block(r0, r1)

            softmax_pass(with_s=True, last=last)

    _legalize_matmul_waits(nc, mybir)
    return nc


def _legalize_matmul_waits(nc, mybir, max_waits=2):
    """TRN2 ISA sync-wait structs hold few waits per instruction (2 for PE
    matmult/NoOp, 1 for DVE TensorTensor, ...); codegen aborts on more.
    Move excess waits onto InstNoOps (1 wait each) inserted right before
    on the same engine (adjacent => identical blocking semantics)."""
    cap = {}
    for f in nc.m.functions:
        for blk in f.blocks:
            insts = blk.instructions
            out = []
            changed = False
            for i in insts:
                si = getattr(i, "sync_info", None)
                eng = getattr(i, "engine", None)
                max_waits = cap.get(type(i).__name__, 1)
                if (si is not None and eng is not None
                        and len(si.on_wait) > max_waits):
                    waits = list(si.on_wait)
                    keep, move = [], []
                    for w in waits:
                        if "PE" in w.ant_name and len(keep) < max_waits:
                            keep.append(w)
                        else:
                            move.append(w)
                    while len(keep) < max_waits and move:
                        keep.append(move.pop())
                    nop_cap = cap.get("InstNoOp", 1)
                    while move:
                        grp, move = move[:nop_cap], move[nop_cap:]
                        nop = mybir.InstNoOp(
                            name=nc.get_next_instruction_name(),
                            engine=eng, ins=[], outs=[])
                        nop.sync_info = mybir.SyncInfo(on_wait=grp,
                                                       on_update=[])
                        out.append(nop)
                    i.sync_info = mybir.SyncInfo(
                        on_wait=keep, on_update=list(si.on_update))
                    changed = True
                out.append(i)
            if changed:
                blk.instructions = out


def _prep_shards(logits, img, compat):
    """Host-side shard prep -> list of 8 in_maps."""
    mneg = np.kron(np.eye(G), -compat.T.astype(np.float64)).astype(np.float16)
    mneg2 = ((1.0 + WC) *
             np.kron(np.eye(G), -compat.T.astype(np.float64))
             ).astype(np.float16)
    iden = np.eye(P, dtype=np.float16)
    onesd = np.kron(np.eye(G), np.ones((C, 1))).astype(np.float16)
    bneg = np.kron(np.eye(G), -np.ones((1, C))).astype(np.float16)
    m18 = np.kron(np.eye(G), np.ones((3, C))).astype(np.float16)
    lnsw = np.zeros((P, 13), np.float32)
    for ki, (dy, dx) in enumerate(TAPS):
        lnsw[:, ki] = math.log(SW[2 + dy, 2 + dx])
    lnsw[:, 12] = -LN21

    in_maps = []
    for core in range(8):
        b, j = divmod(core, 4)
        s = STARTS[j]
        lg = logits[b, :, s:s + 84, :].reshape(C, G, RG, W)
        lg = np.ascontiguousarray(
            lg.transpose(1, 0, 2, 3).reshape(P, F)).astype(np.float16)
        im = np.zeros((G, 3, IU, IV), np.float16)
        for g in range(G):
            base = s + g * RG - 4
            u0, u1 = max(0, -base), min(IU, H - base)
            im[g, :, u0:u1, 4:4 + W] = img[b, :, base + u0:base + u1, :]
        im = im.reshape(18, IU * IV)
        in_maps.append({
            "lg": lg, "img": np.ascontiguousarray(im),
            "mneg": mneg, "mneg2": mneg2, "iden": iden, "onesd": onesd,
            "bneg": bneg, "m18": m18, "lnsw": lnsw,
        })
    return in_maps


def kernel(**inputs):
    logits = np.asarray(inputs["logits"], dtype=np.float32)
    img = np.asarray(inputs["img"], dtype=np.float32)
    compat = np.asarray(inputs["compat_mat"], dtype=np.float32)

    from concourse.bass_utils import run_bass_kernel_spmd

    if "nc" not in _BASS_CACHE:
        _BASS_CACHE["nc"] = _build_bass()
    nc = _BASS_CACHE["nc"]

    in_maps = _prep_shards(logits, img, compat)
    res = run_bass_kernel_spmd(nc, in_maps, core_ids=list(range(8)))
    _BASS_CACHE["last_result"] = res

    out = np.zeros((B, C, H, W), np.float32)
    for core in range(8):
        b, j = divmod(core, 4)
        s = STARTS[j]
        lo, hi = OWN[j]
        qc = res.results[core]["qout"].astype(np.float32).reshape(G, C, RG, W)
        qc = qc.transpose(1, 0, 2, 3).reshape(C, 84, W)
        out[b, :, s + lo:s + hi, :] = qc[:, lo:hi, :]
    return out


# revision 20
# speedup vs baseline: 1.0364x; 1.0190x over previous
"""CRF-as-RNN mean-field kernel for Trainium2 (Bass/Tile), 8-core SPMD.

Strategy:
- Shard 2 images x 4 row-strips across 8 cores. Each core gets 84 rows
  (64 owned + halo); 5 mean-field iterations shrink the valid region by
  2 rows/iter, so no inter-core communication is needed at all.
- On-chip layout: partitions = 6 row-groups x 21 channels = 126; free dim
  = 14 rows x 256 cols (+2-row/-col halos for in-tile shifted reads:
  q has 18 row-slots x 260 col-slots; w maps 16 row-slots). Image-boundary
  zero padding is realized by statically-zero halo slots; intra-core group
  halos are refreshed once per iteration with two SBUF->SBUF DMAs.
- The 5x5 spatial gaussian (sigma=0.1) is a numerical delta in f32, so
  sp == q; it is folded into the compat matmul: z += (1+wc)*mneg @ q.
- Bilateral 24-tap MAC on DVE in fp16 2x mode; 12 unique weight maps
  serve opposite tap pairs by symmetry. (GPSIMD offload was tried and
  reverted: concurrent GPSIMD+DVE streams contend ~4x on SBUF.)
- Bilateral is computed in two row-blocks (rows 0-5, rows 6-13) and the
  softmax is chunked [0,1,2,6,3,4,5] so the next iteration's block A
  only waits on chunks {3,6} + halo DMAs; softmax chunks 4,5 and 0..2
  overlap the bilateral DVE burst.
- w-map precompute: img arrives f16; diff/square on DVE at 2x; a single
  [18->126] f16 mask matmul kron(I6, ones(3,21)) does the 3-channel
  reduction AND the 21-channel broadcast in one op; ACT exp applies
  scale=-50 and folds the spatial weight via bias=ln(s_k).
- Softmax chunked through PSUM, all-f16 matmuls; lnD recentered by
  ln(21) (scale=1/21) so f16 holds it accurately; -lnD broadcast back
  into PSUM by a mask matmul; final exp carries bias=-ln(21).
  Output written f16, host upcasts.
"""

import math
import sys
from contextlib import ExitStack

import numpy as np

sys.path.insert(0, "/opt/trn_rl_repo")

# ---------------- problem constants (hardcoded per contract) ----------------
B, C, H, W = 2, 21, 256, 256
G, RG = 6, 14                  # row groups per strip, rows per group
P = G * C                      # 126 partitions
F = RG * W                     # 3584 free elems (real pixels per partition)
NT, NV = 18, 260               # q tile row slots (-2..15), col slots (-2..257)
NTW = 16                       # w/diff/sq tile row slots (-2..13)
IU, IV = 22, 264               # img tile row slots (-4..17), col slots (-4..259)
STARTS = [0, 54, 118, 172]     # strip start rows
OWN = [(0, 64), (10, 74), (10, 74), (20, 84)]  # owned local-row range per strip
NUM_ITERS = 5
NCH, CH = 7, 512               # softmax chunks (512 px = 2 rows)
CHUNK_ORDER = [0, 1, 2, 6, 3, 4, 5]
FW = NTW * NV                  # 4160 w-map free elems
PRE_CHUNKS = [(i * 512, 512) for i in range(8)] + [(4096, 64)]
LN21 = math.log(21.0)
ROW_BLOCKS = [(0, 6), (6, 14)]  # bilateral row blocks
# tap-instances whose product skips the DVE add chain and is instead
# accumulated into PSUM by an extra mneg@prod matmul per softmax chunk
# (chosen among qdy>0 instances, which are dependency-blocked early anyway)
PEI = (4, 6, 8, 10, 12, 16, 20, 22)

# spatial gaussian (5x5, sigma=5), normalized
_ax = np.arange(5, dtype=np.float64) - 2
_xx, _yy = np.meshgrid(_ax, _ax, indexing="ij")
_g = np.exp(-(_xx**2 + _yy**2) / (2 * 5.0**2))
SW = (_g / _g.sum()).astype(np.float64)
WC = float(SW[2, 2])           # center weight (spatial only; color=1 at center)
# 12 unique taps (positive half-window); opposite taps share weight maps
TAPS = [(0, 1), (0, 2), (1, -2), (1, -1), (1, 0), (1, 1), (1, 2),
        (2, -2), (2, -1), (2, 0), (2, 1), (2, 2)]

_BASS_CACHE = {}


def _build_bass():
    import concourse.bass as bass
    import concourse.mybir as mybir
    from concourse import tile

    f32 = mybir.dt.float32
    f16 = mybir.dt.float16
    AF = mybir.ActivationFunctionType

    nc = bass.Bass("TRN2", target_bir_lowering=False, debug=False,
                   enable_asserts=False)

    lg_d = nc.dram_tensor("lg", [P, F], f16, kind="ExternalInput")
    img_d = nc.dram_tensor("img", [18, IU * IV], f16, kind="ExternalInput")
    mneg_d = nc.dram_tensor("mneg", [P, P], f16, kind="ExternalInput")
    mneg2_d = nc.dram_tensor("mneg2", [P, P], f16, kind="ExternalInput")
    iden_d = nc.dram_tensor("iden", [P, P], f16, kind="ExternalInput")
    onesd_d = nc.dram_tensor("onesd", [P, G], f16, kind="ExternalInput")
    bneg_d = nc.dram_tensor("bneg", [G, P], f16, kind="ExternalInput")
    m18_d = nc.dram_tensor("m18", [18, P], f16, kind="ExternalInput")
    lnsw_d = nc.dram_tensor("lnsw", [P, 13], f32, kind="ExternalInput")
    qout_d = nc.dram_tensor("qout", [P, F], f16, kind="ExternalOutput")

    with tile.TileContext(nc) as tc, ExitStack() as ctx:
        const_pool = ctx.enter_context(tc.tile_pool(name="const", bufs=1))
        main_pool = ctx.enter_context(tc.tile_pool(name="main", bufs=1))
        w_pool = ctx.enter_context(tc.tile_pool(name="wmaps", bufs=1))

        pre_pool = ctx.enter_context(tc.tile_pool(name="pre0", bufs=1))
        img_t = pre_pool.tile([18, IU * IV], f16, tag="img")
        nc.sync.dma_start(img_t[:], img_d.ap())
        lg_t = pre_pool.tile([P, F], f16, tag="lg")
        nc.sync.dma_start(lg_t[:], lg_d.ap())
        mneg_t = const_pool.tile([P, P], f16, tag="mneg")
        nc.sync.dma_start(mneg_t[:], mneg_d.ap())
        mneg2_t = const_pool.tile([P, P], f16, tag="mneg2")
        nc.sync.dma_start(mneg2_t[:], mneg2_d.ap())
        iden_t = const_pool.tile([P, P], f16, tag="iden")
        nc.sync.dma_start(iden_t[:], iden_d.ap())
        onesd_t = const_pool.tile([P, G], f16, tag="onesd")
        nc.sync.dma_start(onesd_t[:], onesd_d.ap())
        bneg_t = const_pool.tile([G, P], f16, tag="bneg")
        nc.sync.dma_start(bneg_t[:], bneg_d.ap())
        m18_t = const_pool.tile([18, P], f16, tag="m18")
        nc.sync.dma_start(m18_t[:], m18_d.ap())
        lnsw_t = const_pool.tile([P, 13], f32, tag="lnsw")
        nc.sync.dma_start(lnsw_t[:], lnsw_d.ap())

        # Absorber matmuls: each PE matmul can carry only ~1 sync wait
        # beyond its own-engine wait, so pre-observe every stationary's DMA
        # queue with a 2-column dummy matmul (self-referential rhs => the
        # dummy itself waits on exactly one DMA sem).
        with tc.tile_pool(name="scrp", bufs=1, space="PSUM") as scrp:
            scr = scrp.tile([G, 2], f32, tag="scr")
            nc.tensor.matmul(scr[:1, :], mneg_t[:, 0:1], mneg_t[:, 0:2],
                             start=True, stop=True)
            nc.tensor.matmul(scr[:1, :], mneg2_t[:, 0:1], mneg2_t[:, 0:2],
                             start=True, stop=True)
            nc.tensor.matmul(scr[:1, :], iden_t[:, 0:1], iden_t[:, 0:2],
                             start=True, stop=True)
            nc.tensor.matmul(scr[:, :], onesd_t[:], onesd_t[:, 0:2],
                             start=True, stop=True)
            nc.tensor.matmul(scr[:1, :], bneg_t[:, 0:1], bneg_t[:, 0:2],
                             start=True, stop=True)
            nc.tensor.matmul(scr[:1, :], m18_t[:, 0:1], m18_t[:, 0:2],
                             start=True, stop=True)

        q_t = main_pool.tile([P, NT * NV], f16, tag="q")
        q3 = q_t[:].rearrange("p (t v) -> p t v", v=NV)
        # only halo slots need zeroing; the interior is written by the
        # initial softmax before any bilateral read
        nc.vector.memset(q3[:, 0:2, 0:NV], 0.0)
        nc.vector.memset(q3[:, 16:18, 0:NV], 0.0)
        nc.vector.memset(q3[:, 2:16, 0:2], 0.0)
        nc.vector.memset(q3[:, 2:16, 258:260], 0.0)

        w_tiles = [w_pool.tile([P, FW], f16, tag=f"w{i}", name=f"w{i}")
                   for i in range(len(TAPS))]

        zps_pool = ctx.enter_context(tc.tile_pool(name="zps", bufs=3,
                                                  space="PSUM"))
        dps_pool = ctx.enter_context(tc.tile_pool(name="dps", bufs=2,
                                                  space="PSUM"))

        # ---------------- iteration tiles ----------------
        post_pool = ctx.enter_context(tc.tile_pool(name="post", bufs=1))
        acca_t = post_pool.tile([P, F], f16, tag="acca")
        acca3 = acca_t[:].rearrange("p (r x) -> p r x", x=W)
        qo3 = acca3  # final pass writes into acca (dead by then)
        prod_ts = []  # filled after the precompute pool closes (reuses SBUF)
        tmp_pool = ctx.enter_context(tc.tile_pool(name="tmp", bufs=3))
        e_pool = ctx.enter_context(tc.tile_pool(name="E", bufs=2))
        ln_pool = ctx.enter_context(tc.tile_pool(name="ln", bufs=2))

        def sm_chunk(c, with_s, last):
            use_prods = with_s
            sl = slice(c * CH, (c + 1) * CH)
            z_ps = zps_pool.tile([P, CH], f32, tag="z")
            if with_s:
                nc.tensor.matmul(z_ps[:], mneg_t[:], acca_t[:, sl],
                                 start=True, stop=False)
                if use_prods:
                    for pt in prod_ts:
                        nc.tensor.matmul(z_ps[:], mneg_t[:], pt[:, sl],
                                         start=False, stop=False,
                                         skip_group_check=True)
                nc.tensor.matmul(z_ps[:], mneg2_t[:],
                                 q3[:, 2 + 2 * c:4 + 2 * c, 2:2 + W],
                                 start=False, stop=False,
                                 skip_group_check=True)
                nc.tensor.matmul(z_ps[:], iden_t[:], lg_t[:, sl],
                                 start=False, stop=False,
                                 skip_group_check=True)
            else:
                nc.tensor.matmul(z_ps[:], iden_t[:], lg_t[:, sl],
                                 start=True, stop=False,
                                 skip_group_check=True)
            e_t = e_pool.tile([P, CH], f16, tag="E")
            nc.scalar.activation(e_t[:], z_ps[:], AF.Exp)
            d_ps = dps_pool.tile([G, CH], f32, tag="D")
            nc.tensor.matmul(d_ps[:], onesd_t[:], e_t[:],
                             start=True, stop=True)
            ln_t = ln_pool.tile([G, CH], f16, tag="ln")
            # ln(D/21) stays ~O(1) => accurate in f16
            nc.scalar.activation(ln_t[:], d_ps[:], AF.Ln,
                                 scale=float(1.0 / 21.0))
            nc.tensor.matmul(z_ps[:], bneg_t[:], ln_t[:],
                             start=False, stop=True,
                             skip_group_check=True)
            z3 = z_ps[:].rearrange("p (r x) -> p r x", x=W)
            if last:
                nc.scalar.activation(qo3[:, 2 * c:2 * c + 2, 0:W],
                                     z3, AF.Exp, bias=lnsw_t[:, 12:13])
                nc.sync.dma_start(qout_d.ap()[:, sl], acca_t[:, sl])
            else:
                nc.scalar.activation(
                    q3[:, 2 + 2 * c:4 + 2 * c, 2:2 + W], z3, AF.Exp,
                    bias=lnsw_t[:, 12:13])

        def softmax_pass(with_s, last):
            for c in CHUNK_ORDER:
                sm_chunk(c, with_s, last)

        def bilateral_block(r0, r1):
            rw = (r1 - r0) * W
            fsl = slice(r0 * W, r1 * W)
            first_d = True
            idx = 0
            for ki, (dy, dx) in enumerate(TAPS):
                w3 = w_tiles[ki][:].rearrange("p (t v) -> p t v", v=NV)
                for (qdy, qdx, wdy, wdx) in ((dy, dx, 0, 0),
                                             (-dy, -dx, -dy, -dx)):
                    q_ap = q3[:, 2 + qdy + r0:2 + qdy + r1,
                              2 + qdx:2 + qdx + W]
                    w_ap = w3[:, 2 + wdy + r0:2 + wdy + r1,
                              2 + wdx:2 + wdx + W]
                    if idx in PEI:
                        pt = prod_ts[PEI.index(idx)]
                        p3 = pt[:].rearrange("p (r x) -> p r x", x=W)
                        nc.vector.tensor_mul(p3[:, r0:r1, 0:W], q_ap, w_ap)
                    elif first_d:
                        nc.vector.tensor_mul(acca3[:, r0:r1, 0:W],
                                             q_ap, w_ap)
                        first_d = False
                    else:
                        t = tmp_pool.tile([P, 8 * W], f16, tag="t")
                        t3 = t[:, 0:rw].rearrange("p (r x) -> p r x", x=W)
                        nc.vector.tensor_mul(t3, q_ap, w_ap)
                        nc.vector.tensor_add(acca_t[:, fsl],
                                             acca_t[:, fsl], t[:, 0:rw])
                    idx += 1

        # ---------------- w-map precompute (init softmax issued after tap 1
        # so tap-0/1 PE matmuls recycle the diff buffers quickly; the PE/ACT
        # precompute stream hides under DVE sub/square work and the first
        # iteration's bilateral; tap ki only needs w_ki) ----------
        with tc.tile_pool(name="pre", bufs=2) as prep, \
             tc.tile_pool(name="psp", bufs=3, space="PSUM") as psp:
            img3 = img_t[:].rearrange("p (u v) -> p u v", v=IV)

            for ki, (dy, dx) in enumerate(TAPS):
                if ki == 2:
                    softmax_pass(with_s=False, last=False)  # q0
                diff_t = prep.tile([18, FW], f16, tag="diff")
                diff3 = diff_t[:].rearrange("p (t v) -> p t v", v=NV)
                nc.vector.tensor_sub(
                    diff3[:, 0:NTW, 0:NV],
                    img3[:, 2 + dy:2 + dy + NTW, 2 + dx:2 + dx + NV],
                    img3[:, 2:2 + NTW, 2:2 + NV],
                )
                nc.vector.tensor_mul(diff_t[:], diff_t[:], diff_t[:])
                for c0, cw in PRE_CHUNKS:
                    sl = slice(c0, c0 + cw)
                    d2_ps = psp.tile([P, 512], f32, tag="d2")
                    nc.tensor.matmul(d2_ps[:, 0:cw], m18_t[:], diff_t[:, sl],
                                     start=True, stop=True)
                    nc.scalar.activation(w_tiles[ki][:, sl], d2_ps[:, 0:cw],
                                         AF.Exp, scale=-50.0,
                                         bias=lnsw_t[:, ki:ki + 1])

        post2_pool = ctx.enter_context(tc.tile_pool(name="post2", bufs=1))
        prod_ts.extend(
            post2_pool.tile([P, F], f16, tag=f"prod{j}", name=f"prod{j}")
            for j in range(len(PEI)))


        for it in range(NUM_ITERS):
            last = it == NUM_ITERS - 1
            # refresh intra-core group halos (2 SBUF->SBUF DMAs)
            nc.sync.dma_start(q3[21:126, 0:2, 0:NV], q3[0:105, 14:16, 0:NV])
            nc.sync.dma_start(q3[0:105, 16:18, 0:NV], q3[21:126, 2:4, 0:NV])

            for r0, r1 in ROW_BLOCKS:
                bilateral_block(r0, r1)

            softmax_pass(with_s=True, last=last)

    _legalize_matmul_waits(nc, mybir)
    return nc


def _legalize_matmul_waits(nc, mybir, max_waits=2):
    """TRN2 ISA sync-wait structs hold few waits per instruction (2 for PE
    matmult/NoOp, 1 for DVE TensorTensor, ...); codegen aborts on more.
    Move excess waits onto InstNoOps (1 wait each) inserted right before
    on the same engine (adjacent => identical blocking semantics)."""
    cap = {}
    for f in nc.m.functions:
        for blk in f.blocks:
            insts = blk.instructions
            out = []
            changed = False
            for i in insts:
                si = getattr(i, "sync_info", None)
                eng = getattr(i, "engine", None)
                max_waits = cap.get(type(i).__name__, 1)
                if (si is not None and eng is not None
                        and len(si.on_wait) > max_waits):
                    waits = list(si.on_wait)
                    keep, move = [], []
                    for w in waits:
                        if "PE" in w.ant_name and len(keep) < max_waits:
                            keep.append(w)
                        else:
                            move.append(w)
                    while len(keep) < max_waits and move:
                        keep.append(move.pop())
                    nop_cap = cap.get("InstNoOp", 1)
                    while move:
                        grp, move = move[:nop_cap], move[nop_cap:]
                        nop = mybir.InstNoOp(
                            name=nc.get_next_instruction_name(),
                            engine=eng, ins=[], outs=[])
                        nop.sync_info = mybir.SyncInfo(on_wait=grp,
                                                       on_update=[])
                        out.append(nop)
                    i.sync_info = mybir.SyncInfo(
                        on_wait=keep, on_update=list(si.on_update))
                    changed = True
                out.append(i)
            if changed:
                blk.instructions = out


def _prep_shards(logits, img, compat):
    """Host-side shard prep -> list of 8 in_maps."""
    mneg = np.kron(np.eye(G), -compat.T.astype(np.float64)).astype(np.float16)
    mneg2 = ((1.0 + WC) *
             np.kron(np.eye(G), -compat.T.astype(np.float64))
             ).astype(np.float16)
    iden = np.eye(P, dtype=np.float16)
    onesd = np.kron(np.eye(G), np.ones((C, 1))).astype(np.float16)
    bneg = np.kron(np.eye(G), -np.ones((1, C))).astype(np.float16)
    m18 = np.kron(np.eye(G), np.ones((3, C))).astype(np.float16)
    lnsw = np.zeros((P, 13), np.float32)
    for ki, (dy, dx) in enumerate(TAPS):
        lnsw[:, ki] = math.log(SW[2 + dy, 2 + dx])
    lnsw[:, 12] = -LN21

    in_maps = []
    for core in range(8):
        b, j = divmod(core, 4)
        s = STARTS[j]
        lg = logits[b, :, s:s + 84, :].reshape(C, G, RG, W)
        lg = np.ascontiguousarray(
            lg.transpose(1, 0, 2, 3).reshape(P, F)).astype(np.float16)
        im = np.zeros((G, 3, IU, IV), np.float16)
        for g in range(G):
            base = s + g * RG - 4
            u0, u1 = max(0, -base), min(IU, H - base)
            im[g, :, u0:u1, 4:4 + W] = img[b, :, base + u0:base + u1, :]
        im = im.reshape(18, IU * IV)
        in_maps.append({
            "lg": lg, "img": np.ascontiguousarray(im),
            "mneg": mneg, "mneg2": mneg2, "iden": iden, "onesd": onesd,
            "bneg": bneg, "m18": m18, "lnsw": lnsw,
        })
    return in_maps


def kernel(**inputs):
    logits = np.asarray(inputs["logits"], dtype=np.float32)
    img = np.asarray(inputs["img"], dtype=np.float32)
    compat = np.asarray(inputs["compat_mat"], dtype=np.float32)

    from concourse.bass_utils import run_bass_kernel_spmd

    if "nc" not in _BASS_CACHE:
        _BASS_CACHE["nc"] = _build_bass()
    nc = _BASS_CACHE["nc"]

    in_maps = _prep_shards(logits, img, compat)
    res = run_bass_kernel_spmd(nc, in_maps, core_ids=list(range(8)))
    _BASS_CACHE["last_result"] = res

    out = np.zeros((B, C, H, W), np.float32)
    for core in range(8):
        b, j = divmod(core, 4)
        s = STARTS[j]
        lo, hi = OWN[j]
        qc = res.results[core]["qout"].astype(np.float32).reshape(G, C, RG, W)
        qc = qc.transpose(1, 0, 2, 3).reshape(C, 84, W)
        out[b, :, s + lo:s + hi, :] = qc[:, lo:hi, :]
    return out


# revision 22
# speedup vs baseline: 1.0789x; 1.0410x over previous
"""CRF-as-RNN mean-field kernel for Trainium2 (Bass/Tile), 8-core SPMD.

Strategy:
- Shard 2 images x 4 row-strips across 8 cores. Each core gets 84 rows
  (64 owned + halo); 5 mean-field iterations shrink the valid region by
  2 rows/iter, so no inter-core communication is needed at all.
- On-chip layout: partitions = 6 row-groups x 21 channels = 126; free dim
  = 14 rows x 256 cols (+2-row/-col halos for in-tile shifted reads:
  q has 18 row-slots x 260 col-slots; w maps 16 row-slots). Image-boundary
  zero padding is realized by statically-zero halo slots; intra-core group
  halos are refreshed once per iteration with two SBUF->SBUF DMAs.
- The 5x5 spatial gaussian (sigma=0.1) is a numerical delta in f32, so
  sp == q; it is folded into the compat matmul: z += (1+wc)*mneg @ q.
- Bilateral 24-tap MAC on DVE in fp16 2x mode; 12 unique weight maps
  serve opposite tap pairs by symmetry. (GPSIMD offload was tried and
  reverted: concurrent GPSIMD+DVE streams contend ~4x on SBUF.)
- Bilateral is computed in two row-blocks (rows 0-5, rows 6-13) and the
  softmax is chunked [0,1,2,6,3,4,5] so the next iteration's block A
  only waits on chunks {3,6} + halo DMAs; softmax chunks 4,5 and 0..2
  overlap the bilateral DVE burst.
- w-map precompute: img arrives f16; diff/square on DVE at 2x; a single
  [18->126] f16 mask matmul kron(I6, ones(3,21)) does the 3-channel
  reduction AND the 21-channel broadcast in one op; ACT exp applies
  scale=-50 and folds the spatial weight via bias=ln(s_k).
- Softmax chunked through PSUM, all-f16 matmuls; lnD recentered by
  ln(21) (scale=1/21) so f16 holds it accurately; -lnD broadcast back
  into PSUM by a mask matmul; final exp carries bias=-ln(21).
  Output written f16, host upcasts.
"""

import math
import sys
from contextlib import ExitStack

import numpy as np

sys.path.insert(0, "/opt/trn_rl_repo")

# ---------------- problem constants (hardcoded per contract) ----------------
B, C, H, W = 2, 21, 256, 256
G, RG = 6, 14                  # row groups per strip, rows per group
P = G * C                      # 126 partitions
F = RG * W                     # 3584 free elems (real pixels per partition)
NT, NV = 18, 260               # q tile row slots (-2..15), col slots (-2..257)
NTW = 16                       # w/diff/sq tile row slots (-2..13)
IU, IV = 22, 264               # img tile row slots (-4..17), col slots (-4..259)
STARTS = [0, 54, 118, 172]     # strip start rows
OWN = [(0, 64), (10, 74), (10, 74), (20, 84)]  # owned local-row range per strip
NUM_ITERS = 5
NCH, CH = 7, 512               # softmax chunks (512 px = 2 rows)
CHUNK_ORDER = [0, 1, 2, 6, 3, 4, 5]
FW = NTW * NV                  # 4160 w-map free elems
PRE_CHUNKS = [(i * 512, 512) for i in range(8)] + [(4096, 64)]
LN21 = math.log(21.0)
ROW_BLOCKS = [(0, 6), (6, 14)]  # bilateral row blocks
# tap-instances whose product skips the DVE add chain and is instead
# accumulated into PSUM by an extra mneg@prod matmul per softmax chunk
# (chosen among qdy>0 instances, which are dependency-blocked early anyway)
PEI = (4, 6, 8, 10, 12, 14, 16, 18, 20, 22)

# spatial gaussian (5x5, sigma=5), normalized
_ax = np.arange(5, dtype=np.float64) - 2
_xx, _yy = np.meshgrid(_ax, _ax, indexing="ij")
_g = np.exp(-(_xx**2 + _yy**2) / (2 * 5.0**2))
SW = (_g / _g.sum()).astype(np.float64)
WC = float(SW[2, 2])           # center weight (spatial only; color=1 at center)
# 12 unique taps (positive half-window); opposite taps share weight maps
TAPS = [(0, 1), (0, 2), (1, -2), (1, -1), (1, 0), (1, 1), (1, 2),
        (2, -2), (2, -1), (2, 0), (2, 1), (2, 2)]

_BASS_CACHE = {}


def _build_bass():
    import concourse.bass as bass
    import concourse.mybir as mybir
    from concourse import tile

    f32 = mybir.dt.float32
    f16 = mybir.dt.float16
    AF = mybir.ActivationFunctionType

    nc = bass.Bass("TRN2", target_bir_lowering=False, debug=False,
                   enable_asserts=False)

    lg_d = nc.dram_tensor("lg", [P, F], f16, kind="ExternalInput")
    img_d = nc.dram_tensor("img", [18, IU * IV], f16, kind="ExternalInput")
    mneg_d = nc.dram_tensor("mneg", [P, P], f16, kind="ExternalInput")
    mneg2_d = nc.dram_tensor("mneg2", [P, P], f16, kind="ExternalInput")
    iden_d = nc.dram_tensor("iden", [P, P], f16, kind="ExternalInput")
    onesd_d = nc.dram_tensor("onesd", [P, G], f16, kind="ExternalInput")
    bneg_d = nc.dram_tensor("bneg", [G, P], f16, kind="ExternalInput")
    m18_d = nc.dram_tensor("m18", [18, P], f16, kind="ExternalInput")
    lnsw_d = nc.dram_tensor("lnsw", [P, 13], f32, kind="ExternalInput")
    qout_d = nc.dram_tensor("qout", [P, F], f16, kind="ExternalOutput")

    with tile.TileContext(nc) as tc, ExitStack() as ctx:
        const_pool = ctx.enter_context(tc.tile_pool(name="const", bufs=1))
        main_pool = ctx.enter_context(tc.tile_pool(name="main", bufs=1))
        w_pool = ctx.enter_context(tc.tile_pool(name="wmaps", bufs=1))

        pre_pool = ctx.enter_context(tc.tile_pool(name="pre0", bufs=1))
        lg_t = pre_pool.tile([P, F], f16, tag="lg")
        nc.sync.dma_start(lg_t[:], lg_d.ap())
        mneg_t = const_pool.tile([P, P], f16, tag="mneg")
        nc.sync.dma_start(mneg_t[:], mneg_d.ap())
        mneg2_t = const_pool.tile([P, P], f16, tag="mneg2")
        nc.sync.dma_start(mneg2_t[:], mneg2_d.ap())
        iden_t = const_pool.tile([P, P], f16, tag="iden")
        nc.sync.dma_start(iden_t[:], iden_d.ap())
        onesd_t = const_pool.tile([P, G], f16, tag="onesd")
        nc.sync.dma_start(onesd_t[:], onesd_d.ap())
        bneg_t = const_pool.tile([G, P], f16, tag="bneg")
        nc.sync.dma_start(bneg_t[:], bneg_d.ap())
        m18_t = const_pool.tile([18, P], f16, tag="m18")
        nc.sync.dma_start(m18_t[:], m18_d.ap())
        lnsw_t = const_pool.tile([P, 13], f32, tag="lnsw")
        nc.sync.dma_start(lnsw_t[:], lnsw_d.ap())

        # Absorber matmuls: each PE matmul can carry only ~1 sync wait
        # beyond its own-engine wait, so pre-observe every stationary's DMA
        # queue with a 2-column dummy matmul (self-referential rhs => the
        # dummy itself waits on exactly one DMA sem).
        with tc.tile_pool(name="scrp", bufs=1, space="PSUM") as scrp:
            scr = scrp.tile([G, 2], f32, tag="scr")
            nc.tensor.matmul(scr[:1, :], mneg_t[:, 0:1], mneg_t[:, 0:2],
                             start=True, stop=True)
            nc.tensor.matmul(scr[:1, :], mneg2_t[:, 0:1], mneg2_t[:, 0:2],
                             start=True, stop=True)
            nc.tensor.matmul(scr[:1, :], iden_t[:, 0:1], iden_t[:, 0:2],
                             start=True, stop=True)
            nc.tensor.matmul(scr[:, :], onesd_t[:], onesd_t[:, 0:2],
                             start=True, stop=True)
            nc.tensor.matmul(scr[:1, :], bneg_t[:, 0:1], bneg_t[:, 0:2],
                             start=True, stop=True)
            nc.tensor.matmul(scr[:1, :], m18_t[:, 0:1], m18_t[:, 0:2],
                             start=True, stop=True)

        q_t = main_pool.tile([P, NT * NV], f16, tag="q")
        q3 = q_t[:].rearrange("p (t v) -> p t v", v=NV)
        # only halo slots need zeroing; the interior is written by the
        # initial softmax before any bilateral read
        nc.vector.memset(q3[:, 0:2, 0:NV], 0.0)
        nc.vector.memset(q3[:, 16:18, 0:NV], 0.0)
        nc.vector.memset(q3[:, 2:16, 0:2], 0.0)
        nc.vector.memset(q3[:, 2:16, 258:260], 0.0)

        w_tiles = [w_pool.tile([P, FW], f16, tag=f"w{i}", name=f"w{i}")
                   for i in range(len(TAPS))]

        zps_pool = ctx.enter_context(tc.tile_pool(name="zps", bufs=3,
                                                  space="PSUM"))
        dps_pool = ctx.enter_context(tc.tile_pool(name="dps", bufs=2,
                                                  space="PSUM"))

        # ---------------- iteration tiles ----------------
        post_pool = ctx.enter_context(tc.tile_pool(name="post", bufs=1))
        acca_t = post_pool.tile([P, F], f16, tag="acca")
        acca3 = acca_t[:].rearrange("p (r x) -> p r x", x=W)
        qo3 = acca3  # final pass writes into acca (dead by then)
        prod_ts = []  # filled after the precompute pool closes (reuses SBUF)
        tmp_pool = ctx.enter_context(tc.tile_pool(name="tmp", bufs=2))
        e_pool = ctx.enter_context(tc.tile_pool(name="E", bufs=2))
        ln_pool = ctx.enter_context(tc.tile_pool(name="ln", bufs=2))

        def sm_chunk(c, with_s, last):
            use_prods = with_s
            sl = slice(c * CH, (c + 1) * CH)
            z_ps = zps_pool.tile([P, CH], f32, tag="z")
            if with_s:
                nc.tensor.matmul(z_ps[:], mneg_t[:], acca_t[:, sl],
                                 start=True, stop=False)
                if use_prods:
                    for pt in prod_ts:
                        nc.tensor.matmul(z_ps[:], mneg_t[:], pt[:, sl],
                                         start=False, stop=False,
                                         skip_group_check=True)
                nc.tensor.matmul(z_ps[:], mneg2_t[:],
                                 q3[:, 2 + 2 * c:4 + 2 * c, 2:2 + W],
                                 start=False, stop=False,
                                 skip_group_check=True)
                nc.tensor.matmul(z_ps[:], iden_t[:], lg_t[:, sl],
                                 start=False, stop=False,
                                 skip_group_check=True)
            else:
                nc.tensor.matmul(z_ps[:], iden_t[:], lg_t[:, sl],
                                 start=True, stop=False,
                                 skip_group_check=True)
            e_t = e_pool.tile([P, CH], f16, tag="E")
            nc.scalar.activation(e_t[:], z_ps[:], AF.Exp)
            d_ps = dps_pool.tile([G, CH], f32, tag="D")
            nc.tensor.matmul(d_ps[:], onesd_t[:], e_t[:],
                             start=True, stop=True)
            ln_t = ln_pool.tile([G, CH], f16, tag="ln")
            # ln(D/21) stays ~O(1) => accurate in f16
            nc.scalar.activation(ln_t[:], d_ps[:], AF.Ln,
                                 scale=float(1.0 / 21.0))
            nc.tensor.matmul(z_ps[:], bneg_t[:], ln_t[:],
                             start=False, stop=True,
                             skip_group_check=True)
            z3 = z_ps[:].rearrange("p (r x) -> p r x", x=W)
            if last:
                nc.scalar.activation(qo3[:, 2 * c:2 * c + 2, 0:W],
                                     z3, AF.Exp, bias=lnsw_t[:, 12:13])
                nc.sync.dma_start(qout_d.ap()[:, sl], acca_t[:, sl])
            else:
                nc.scalar.activation(
                    q3[:, 2 + 2 * c:4 + 2 * c, 2:2 + W], z3, AF.Exp,
                    bias=lnsw_t[:, 12:13])

        def softmax_pass(with_s, last):
            for c in CHUNK_ORDER:
                sm_chunk(c, with_s, last)

        def bilateral_block(r0, r1):
            rw = (r1 - r0) * W
            fsl = slice(r0 * W, r1 * W)
            first_d = True
            idx = 0
            for ki, (dy, dx) in enumerate(TAPS):
                w3 = w_tiles[ki][:].rearrange("p (t v) -> p t v", v=NV)
                for (qdy, qdx, wdy, wdx) in ((dy, dx, 0, 0),
                                             (-dy, -dx, -dy, -dx)):
                    q_ap = q3[:, 2 + qdy + r0:2 + qdy + r1,
                              2 + qdx:2 + qdx + W]
                    w_ap = w3[:, 2 + wdy + r0:2 + wdy + r1,
                              2 + wdx:2 + wdx + W]
                    if idx in PEI:
                        pt = prod_ts[PEI.index(idx)]
                        p3 = pt[:].rearrange("p (r x) -> p r x", x=W)
                        nc.vector.tensor_mul(p3[:, r0:r1, 0:W], q_ap, w_ap)
                    elif first_d:
                        nc.vector.tensor_mul(acca3[:, r0:r1, 0:W],
                                             q_ap, w_ap)
                        first_d = False
                    else:
                        t = tmp_pool.tile([P, 8 * W], f16, tag="t")
                        t3 = t[:, 0:rw].rearrange("p (r x) -> p r x", x=W)
                        nc.vector.tensor_mul(t3, q_ap, w_ap)
                        nc.vector.tensor_add(acca_t[:, fsl],
                                             acca_t[:, fsl], t[:, 0:rw])
                    idx += 1

        # ---------------- w-map precompute (init softmax issued after tap 1
        # so tap-0/1 PE matmuls recycle the diff buffers quickly; the PE/ACT
        # precompute stream hides under DVE sub/square work and the first
        # iteration's bilateral; tap ki only needs w_ki) ----------
        img_stack = ExitStack()
        img_pool = img_stack.enter_context(tc.tile_pool(name="img", bufs=1))
        img_t = img_pool.tile([18, IU * IV], f16, tag="img")
        nc.sync.dma_start(img_t[:], img_d.ap())
        with tc.tile_pool(name="pre", bufs=2) as prep, \
             tc.tile_pool(name="psp", bufs=3, space="PSUM") as psp:
            img3 = img_t[:].rearrange("p (u v) -> p u v", v=IV)

            for ki, (dy, dx) in enumerate(TAPS):
                if ki == 2:
                    softmax_pass(with_s=False, last=False)  # q0
                diff_t = prep.tile([18, FW], f16, tag="diff")
                diff3 = diff_t[:].rearrange("p (t v) -> p t v", v=NV)
                nc.vector.tensor_sub(
                    diff3[:, 0:NTW, 0:NV],
                    img3[:, 2 + dy:2 + dy + NTW, 2 + dx:2 + dx + NV],
                    img3[:, 2:2 + NTW, 2:2 + NV],
                )
                nc.vector.tensor_mul(diff_t[:], diff_t[:], diff_t[:])
                for c0, cw in PRE_CHUNKS:
                    sl = slice(c0, c0 + cw)
                    d2_ps = psp.tile([P, 512], f32, tag="d2")
                    nc.tensor.matmul(d2_ps[:, 0:cw], m18_t[:], diff_t[:, sl],
                                     start=True, stop=True)
                    nc.scalar.activation(w_tiles[ki][:, sl], d2_ps[:, 0:cw],
                                         AF.Exp, scale=-50.0,
                                         bias=lnsw_t[:, ki:ki + 1])

        img_stack.close()
        post2_pool = ctx.enter_context(tc.tile_pool(name="post2", bufs=1))
        prod_ts.extend(
            post2_pool.tile([P, F], f16, tag=f"prod{j}", name=f"prod{j}")
            for j in range(len(PEI)))


        for it in range(NUM_ITERS):
            last = it == NUM_ITERS - 1
            # refresh intra-core group halos (2 SBUF->SBUF DMAs)
            nc.sync.dma_start(q3[21:126, 0:2, 0:NV], q3[0:105, 14:16, 0:NV])
            nc.sync.dma_start(q3[0:105, 16:18, 0:NV], q3[21:126, 2:4, 0:NV])

            for r0, r1 in ROW_BLOCKS:
                bilateral_block(r0, r1)

            softmax_pass(with_s=True, last=last)

    _legalize_matmul_waits(nc, mybir)
    return nc


def _legalize_matmul_waits(nc, mybir, max_waits=2):
    """TRN2 ISA sync-wait structs hold few waits per instruction (2 for PE
    matmult/NoOp, 1 for DVE TensorTensor, ...); codegen aborts on more.
    Move excess waits onto InstNoOps (1 wait each) inserted right before
    on the same engine (adjacent => identical blocking semantics)."""
    cap = {}
    for f in nc.m.functions:
        for blk in f.blocks:
            insts = blk.instructions
            out = []
            changed = False
            for i in insts:
                si = getattr(i, "sync_info", None)
                eng = getattr(i, "engine", None)
                max_waits = cap.get(type(i).__name__, 1)
                if (si is not None and eng is not None
                        and len(si.on_wait) > max_waits):
                    waits = list(si.on_wait)
                    keep, move = [], []
                    for w in waits:
                        if "PE" in w.ant_name and len(keep) < max_waits:
                            keep.append(w)
                        else:
                            move.append(w)
                    while len(keep) < max_waits and move:
                        keep.append(move.pop())
                    nop_cap = cap.get("InstNoOp", 1)
                    while move:
                        grp, move = move[:nop_cap], move[nop_cap:]
                        nop = mybir.InstNoOp(
                            name=nc.get_next_instruction_name(),
                            engine=eng, ins=[], outs=[])
                        nop.sync_info = mybir.SyncInfo(on_wait=grp,
                                                       on_update=[])
                        out.append(nop)
                    i.sync_info = mybir.SyncInfo(
                        on_wait=keep, on_update=list(si.on_update))
                    changed = True
                out.append(i)
            if changed:
                blk.instructions = out


def _prep_shards(logits, img, compat):
    """Host-side shard prep -> list of 8 in_maps."""
    mneg = np.kron(np.eye(G), -compat.T.astype(np.float64)).astype(np.float16)
    mneg2 = ((1.0 + WC) *
             np.kron(np.eye(G), -compat.T.astype(np.float64))
             ).astype(np.float16)
    iden = np.eye(P, dtype=np.float16)
    onesd = np.kron(np.eye(G), np.ones((C, 1))).astype(np.float16)
    bneg = np.kron(np.eye(G), -np.ones((1, C))).astype(np.float16)
    m18 = np.kron(np.eye(G), np.ones((3, C))).astype(np.float16)
    lnsw = np.zeros((P, 13), np.float32)
    for ki, (dy, dx) in enumerate(TAPS):
        lnsw[:, ki] = math.log(SW[2 + dy, 2 + dx])
    lnsw[:, 12] = -LN21

    in_maps = []
    for core in range(8):
        b, j = divmod(core, 4)
        s = STARTS[j]
        lg = logits[b, :, s:s + 84, :].reshape(C, G, RG, W)
        lg = np.ascontiguousarray(
            lg.transpose(1, 0, 2, 3).reshape(P, F)).astype(np.float16)
        im = np.zeros((G, 3, IU, IV), np.float16)
        for g in range(G):
            base = s + g * RG - 4
            u0, u1 = max(0, -base), min(IU, H - base)
            im[g, :, u0:u1, 4:4 + W] = img[b, :, base + u0:base + u1, :]
        im = im.reshape(18, IU * IV)
        in_maps.append({
            "lg": lg, "img": np.ascontiguousarray(im),
            "mneg": mneg, "mneg2": mneg2, "iden": iden, "onesd": onesd,
            "bneg": bneg, "m18": m18, "lnsw": lnsw,
        })
    return in_maps


def kernel(**inputs):
    logits = np.asarray(inputs["logits"], dtype=np.float32)
    img = np.asarray(inputs["img"], dtype=np.float32)
    compat = np.asarray(inputs["compat_mat"], dtype=np.float32)

    from concourse.bass_utils import run_bass_kernel_spmd

    if "nc" not in _BASS_CACHE:
        _BASS_CACHE["nc"] = _build_bass()
    nc = _BASS_CACHE["nc"]

    in_maps = _prep_shards(logits, img, compat)
    res = run_bass_kernel_spmd(nc, in_maps, core_ids=list(range(8)))
    _BASS_CACHE["last_result"] = res

    out = np.zeros((B, C, H, W), np.float32)
    for core in range(8):
        b, j = divmod(core, 4)
        s = STARTS[j]
        lo, hi = OWN[j]
        qc = res.results[core]["qout"].astype(np.float32).reshape(G, C, RG, W)
        qc = qc.transpose(1, 0, 2, 3).reshape(C, 84, W)
        out[b, :, s + lo:s + hi, :] = qc[:, lo:hi, :]
    return out
